# revision 20
# baseline (speedup 1.0000x reference)
"""AdaConv Trainium2 kernel — 8-core SPMD, data-parallel over batch.

Per core c (sample c):
  Stage A: kernel-prediction net for ALL 8 samples, layer-2 weights sharded
           by output channel across cores; AllToAll redistributes so each
           core ends with the full dynamic weights for its own sample.
  Stage B: build fused per-sample conv weights W_eff = PK o D and expand
           them into block-diagonal stationary matrices S via PE select
           matmuls + mask.
  Stage C: main grouped conv (128 groups of 4->4 ch, 3x3, reflect pad) as
           9 PSUM-accumulated bf16 matmuls per 128-channel chunk.

Perf structure (v3):
  - All kernel-prediction weights host-packed into 3 bf16 tensors in final
    SBUF layout -> 3 contiguous DMAs (few hundred descriptors) instead of
    thousands of tiny strided ones. Biases/constants packed into one f32
    tensor.
  - The big `predicted` input loads contiguously on the ACT HWDGE ring;
    pad+f32->bf16 convert on-chip (scalar engine), overlapped with the
    collective.
  - AllToAll payload in bf16 with dw+pk interleaved per channel so the
    write is runs-of-10 and the read is one contiguous 40-element run per
    channel.
  - Stage B matmuls in bf16; output stored bf16 (host converts to f32).
"""
import sys
import types

sys.path.insert(0, "/opt/trn_rl_repo")

import numpy as np
import ml_dtypes

import concourse.bass as bass
import concourse.mybir as mybir

N = 8          # batch == cores
CIN = 512
COUT = 512
HW = 64        # spatial
HWP = 66       # padded
NPOS = 16      # style spatial 4x4
OSL = 2048 // N      # dw2 out-channel slice per core (256)
PKSL = 2048 // N     # pk2 slice (256)
PBSL = 512 // N      # pb2 slice (64)
# AllToAll per-rank block: [ch (256): 9 dw + 1 pk | pb 64]
BLK = OSL * 10 + PBSL      # 2624
BPB = OSL * 10             # 2560
AG_SZ = N * BLK            # 20992

# wbA columns (bf16): [st 512 | w1 2048]
OFF_ST = 0
OFF_W1 = 512
WBA_W = 2560
# wbB columns (bf16): [w2 4096]
WBB_W = 16 * OSL           # 4096
# wbC columns (bf16): [pk1 2048 | pb1 2048 | pk2 1024 | pb2 256 | perm 512]
OFF_PK1 = 0
OFF_PB1 = 2048
OFF_PK2 = 4096
OFF_PB2 = 5120
OFF_PERM = 5376
WBC_W = 5888
# cf columns (f32): [b1 4 | b2 2 | bk1 4 | bb1 4 | bk2 2 | bb2 1 | ident 128 | mask 128]
CF_B1 = 0
CF_B2 = 4
CF_BK1 = 6
CF_BB1 = 10
CF_BK2 = 14
CF_BB2 = 16
CF_IDENT = 17
CF_MASK = 145
CF_W = 273

F32 = mybir.dt.float32
BF16 = mybir.dt.bfloat16
BF16_NP = ml_dtypes.bfloat16


# ---------------------------------------------------------------- tile patch
def _install_tile_patch():
    """walrus here rejects Drain instructions with >1 sync-wait; spread the
    Tile tail-drain waits over individual SP nops."""
    import concourse.tile as tile_mod
    from concourse.vector_clock import ScopedClock

    def _patched(self, tick_clock, wait_clock):
        nc = self.nc
        drain_inst = nc.sync.drain()
        wait_clock.add_sem_waits(
            drain_inst.ins, ScopedClock({None: tick_clock.global_clock})
        )
        waits = list(drain_inst.ins.sync_info.on_wait or [])
        if len(waits) > 1:
            drain_inst.ins.sync_info.on_wait = waits[:1]
            for w in waits[1:]:
                nop = nc.sync.nop(nofuse=True, hint="tail_wait_split")
                if nop.ins.sync_info is None:
                    nop.ins.sync_info = mybir.SyncInfo(on_wait=[w], on_update=[])
                else:
                    nop.ins.sync_info.on_wait = [w]
        nc.all_engine_barrier()
        assert self.sems is not None
        popped = nc._tile_sem_poison_stack.pop()
        assert popped is self._sem_poison
        nc.clear_and_free_semaphores(list(self.sems.allocated().values()))
        nc.all_engine_barrier()

    tile_mod.TileContext._drain_and_barrier = _patched


_install_tile_patch()
from concourse.tile import TileContext  # noqa: E402


def install_profile_shim():
    """antenv.axon_hooks is missing from this image; recreate it so
    run_bass_kernel_spmd(trace=True) can capture NTFF profiles."""
    if "antenv.axon_hooks" in sys.modules:
        return
    import antenv

    mod = types.ModuleType("antenv.axon_hooks")
    mod._hook = None
    mod.set_axon_ntff_profile_hook = lambda h: setattr(mod, "_hook", h)
    mod.get_axon_ntff_profile_hook = lambda: mod._hook
    sys.modules["antenv.axon_hooks"] = mod
    antenv.axon_hooks = mod
    try:
        if "/root/.axon_site" not in sys.path:
            sys.path.insert(0, "/root/.axon_site")
        from trn_agent_boot.trn_boot import _ntff_profile_via_ctypes

        hook = _ntff_profile_via_ctypes("/opt/axon/libaxon_pjrt.so")
        mod.set_axon_ntff_profile_hook(hook)
    except Exception:
        pass


def _ap(t_ap, offset, dims):
    """Custom flat AP over a tile's underlying tensor."""
    return bass.AP(t_ap.tensor, offset, [list(d) for d in dims])


def _pt(t):
    """Physical partition pitch (elements) of a tile."""
    return t[:, :].ap[0][0]


def _split_excess_waits(nc, max_waits=1):
    """This walrus build rejects instructions carrying more than ~1 sync-wait.
    Move excess waits onto same-engine NoOps inserted just before."""
    n_split = 0
    for f in nc.m.functions:
        for bb in f.blocks:
            newlist = []
            for inst in bb.instructions:
                si = getattr(inst, "sync_info", None)
                if si is not None and si.on_wait and len(si.on_wait) > max_waits:
                    waits = list(si.on_wait)
                    for k, w in enumerate(waits[max_waits:]):
                        nop = mybir.InstNoOp(
                            name=f"{inst.name}_ws{k}",
                            engine=inst.engine,
                            bass_nofuse=True,
                            sync_info=mybir.SyncInfo(on_wait=[w], on_update=[]),
                        )
                        newlist.append(nop)
                        n_split += 1
                    si.on_wait = waits[:max_waits]
                newlist.append(inst)
            try:
                bb.instructions[:] = newlist
            except TypeError:
                bb.set_instructions(newlist)
    return n_split


def build_nc():
    nc = bass.Bass(target_bir_lowering=False)

    wbA_p = nc.declare_dram_parameter("wbA", [128, WBA_W], BF16, isOutput=False)
    wbB_p = nc.declare_dram_parameter("wbB", [128, WBB_W], BF16, isOutput=False)
    wbC_p = nc.declare_dram_parameter("wbC", [128, WBC_W], BF16, isOutput=False)
    cf_p = nc.declare_dram_parameter("cf", [128, CF_W], F32, isOutput=False)
    selm_p = nc.declare_dram_parameter("selm", [36, 9 * 128], BF16, isOutput=False)
    xin = nc.declare_dram_parameter("xin", [CIN, HW * HW], F32, isOutput=False)
    out = nc.declare_dram_parameter("out", [COUT, HW * HW], BF16, isOutput=True)

    with TileContext(nc) as tc:
        with (
            tc.tile_pool(name="sb", bufs=1) as sb,
            tc.tile_pool(name="sbx", bufs=1) as sbx,
            tc.tile_pool(name="sbo", bufs=2) as sbo,
            tc.tile_pool(name="ps", bufs=2, space="PSUM") as ps,
            tc.tile_pool(name="psc", bufs=4, space="PSUM") as psc,
            tc.tile_pool(name="dram", bufs=1, space="DRAM") as dram,
        ):
            # ================ SP-ring loads: packed weights/constants
            wbA = sb.tile([128, WBA_W], BF16, tag="wbA", name="wbA")
            nc.sync.dma_start(out=wbA[:, :], in_=wbA_p[:, :])
            wbB = sb.tile([128, WBB_W], BF16, tag="wbB", name="wbB")
            nc.sync.dma_start(out=wbB[:, :], in_=wbB_p[:, :])
            wbC = sb.tile([128, WBC_W], BF16, tag="wbC", name="wbC")
            nc.sync.dma_start(out=wbC[:, :], in_=wbC_p[:, :])
            cf = sb.tile([128, CF_W], F32, tag="cf", name="cf")
            nc.sync.dma_start(out=cf[:, :], in_=cf_p[:, :])
            selsb = sb.tile([36, 9 * 128], BF16, tag="selsb", name="selsb")
            nc.sync.dma_start(out=selsb[:, :], in_=selm_p[:, :])
            ptA, ptB, ptC, ptF = _pt(wbA), _pt(wbB), _pt(wbC), _pt(cf)

            def cfb(col, parts=128):
                return _ap(cf, col, [[ptF, parts], [1, 1]])

            # xsb input tiles are loaded on the SYNC ring AFTER the agin
            # writes (below): a dma_start occupies its issuing sequencer
            # until descriptors drain (~6us per 2MB chunk), and the 8MB of
            # Q_X packets would otherwise stall the tiny agin packets that
            # gate the collective. Issued post-agin, they fill the CC
            # firmware+skew dead time instead.
            xsb = [sbx.tile([128, HW * HW], F32, tag=f"xsb{ch}", name=f"xsb{ch}")
                   for ch in range(4)]

            # ================ stage A: layer-1 (h = lrelu(W1 s + b1))
            h = [sb.tile([128, N * NPOS], BF16, tag=f"h{ot}", name=f"h{ot}") for ot in range(4)]
            for ot in range(4):
                hp = ps.tile([128, N * NPOS], F32, tag="sA", name="sA")
                for it in range(4):
                    nc.tensor.matmul(
                        hp[:, :],
                        _ap(wbA, OFF_W1 + it * CIN + ot * 128, [[ptA, 128], [1, 128]]),
                        _ap(wbA, OFF_ST + it * 128, [[ptA, 128], [1, 128]]),
                        start=(it == 0),
                        stop=(it == 3),
                    )
                nc.scalar.activation(
                    h[ot][:, :], hp[:, :], mybir.ActivationFunctionType.Identity,
                    bias=cfb(CF_B1 + ot),
                )
                zt = sb.tile([128, N * NPOS], BF16, tag="zt", name="zt")
                nc.vector.tensor_scalar_mul(zt[:, :], h[ot][:, :], 0.01)
                nc.vector.tensor_max(h[ot][:, :], h[ot][:, :], zt[:, :])

            # ---------------- stage A: dw2 slice for all samples
            agin = dram.tile([AG_SZ], BF16)
            agout = dram.tile([AG_SZ], BF16)
            dppk = [sb.tile([128, 80], BF16, tag=f"dppk{o2}", name=f"dppk{o2}")
                    for o2 in range(2)]
            for o2 in range(2):
                dps = ps.tile([128, N * 9], F32, tag="sA", name="sA")
                for kt in range(16):
                    it, tap = kt // 4, kt % 4
                    di, dj = tap // 2, tap % 2
                    rhs = _ap(h[it], di * 4 + dj,
                              [[_pt(h[it]), 128], [NPOS, N], [4, 3], [1, 3]])
                    nc.tensor.matmul(
                        dps[:, :],
                        _ap(wbB, kt * OSL + o2 * 128, [[ptB, 128], [1, 128]]),
                        rhs,
                        start=(kt == 0),
                        stop=(kt == 15),
                    )
                # dw taps -> cols {n*10+0..8} of dppk
                nc.scalar.activation(
                    _ap(dppk[o2], 0, [[_pt(dppk[o2]), 128], [10, N], [1, 9]]),
                    _ap(dps, 0, [[_pt(dps), 128], [9, N], [1, 9]]),
                    mybir.ActivationFunctionType.Identity,
                    bias=cfb(CF_B2 + o2),
                )

            # ---------------- stage A: pooled-style path (pk / pb)
            sp = [sb.tile([128, N], BF16, tag=f"sp{i}", name=f"sp{i}") for i in range(4)]
            spf = sb.tile([128, N], F32, tag="spf", name="spf")
            for i in range(4):
                nc.vector.tensor_reduce(
                    spf[:, :],
                    _ap(wbA, OFF_ST + i * 128, [[ptA, 128], [NPOS, N], [1, NPOS]]),
                    axis=mybir.AxisListType.X,
                    op=mybir.AluOpType.add,
                )
                nc.vector.tensor_scalar_mul(sp[i][:, :], spf[:, :], 1.0 / NPOS)

            def layer1(off, bcol, tagp):
                acts = []
                for ot in range(4):
                    ap_ = ps.tile([128, N], F32, tag="sA", name="sA")
                    for it in range(4):
                        nc.tensor.matmul(
                            ap_[:, :],
                            _ap(wbC, off + it * CIN + ot * 128, [[ptC, 128], [1, 128]]),
                            sp[it][:, :],
                            start=(it == 0),
                            stop=(it == 3),
                        )
                    a = sb.tile([128, N], BF16, tag=f"{tagp}a{ot}", name=f"{tagp}a{ot}")
                    nc.scalar.activation(
                        a[:, :], ap_[:, :], mybir.ActivationFunctionType.Identity,
                        bias=cfb(bcol + ot),
                    )
                    zt2 = sb.tile([128, N], BF16, tag="zt2", name="zt2")
                    nc.vector.tensor_scalar_mul(zt2[:, :], a[:, :], 0.01)
                    nc.vector.tensor_max(a[:, :], a[:, :], zt2[:, :])
                    acts.append(a)
                return acts

            a1 = layer1(OFF_PK1, CF_BK1, "pk1")
            c1 = layer1(OFF_PB1, CF_BB1, "pb1")

            for o2 in range(2):
                pp = ps.tile([128, N], F32, tag="sA", name="sA")
                for it in range(4):
                    nc.tensor.matmul(
                        pp[:, :],
                        _ap(wbC, OFF_PK2 + it * PKSL + o2 * 128, [[ptC, 128], [1, 128]]),
                        a1[it][:, :],
                        start=(it == 0),
                        stop=(it == 3),
                    )
                # pk -> col {n*10+9} of dppk
                nc.scalar.activation(
                    _ap(dppk[o2], 9, [[_pt(dppk[o2]), 128], [10, N]]),
                    pp[:, :],
                    mybir.ActivationFunctionType.Identity,
                    bias=cfb(CF_BK2 + o2),
                )
                # agin[n*BLK + (o2*128+p)*10 + pos] = dppk[o2][p, n*10+pos]
                nc.sync.dma_start(
                    out=_ap(agin[:], o2 * 128 * 10, [[10, 128], [BLK, N], [1, 10]]),
                    in_=_ap(dppk[o2], 0, [[_pt(dppk[o2]), 128], [1, 80]]),
                )

            pbp = ps.tile([64, N], F32, tag="sA", name="sA")
            for it in range(4):
                nc.tensor.matmul(
                    pbp[:, :],
                    _ap(wbC, OFF_PB2 + it * PBSL, [[ptC, 128], [1, PBSL]]),
                    c1[it][:, :],
                    start=(it == 0),
                    stop=(it == 3),
                )
            pbc = sb.tile([64, 32], BF16, tag="pbc", name="pbc")
            nc.scalar.activation(
                pbc[:, 0:8], pbp[:, :], mybir.ActivationFunctionType.Identity,
                bias=cfb(CF_BB2, 64),
            )
            nc.sync.dma_start(
                out=_ap(agin[:], BPB, [[1, 64], [BLK, N]]),
                in_=_ap(pbc, 0, [[_pt(pbc), 64], [1, N]]),
            )



            # ================ pad + f32->bf16 convert (overlaps collective)
            # All pad work on the scalar (ACT) queue: a late xsb chunk never
            # blocks stage-B/C vector work or the conv PSUM drain.
            xb = [sbx.tile([128, 4384], BF16, tag=f"xb{ch}", name=f"xb{ch}")
                  for ch in range(4)]
            # WAR fences: junk-overwrite the agin source tiles so the scalar
            # queue stalls until the agin DMA reads fully complete. The xsb
            # loads issued next therefore cannot put Q_X packets on the SDMA
            # engines while the tiny agin packets (which gate the collective
            # trigger) are draining.
            for o2 in range(2):
                nc.scalar.activation(
                    dppk[o2][:, 0:1], cf[:, 0:1],
                    mybir.ActivationFunctionType.Copy,
                )
            nc.scalar.activation(
                pbc[:, 8:9], _ap(cf, 0, [[ptF, 64], [1, 1]]),
                mybir.ActivationFunctionType.Copy,
            )
            for ch in range(4):
                nc.scalar.dma_start(
                    out=xsb[ch][:, :], in_=xin[ch * 128:(ch + 1) * 128, :]
                )
                ptx = _pt(xb[ch])
                nc.scalar.activation(
                    _ap(xb[ch], HWP + 1, [[ptx, 128], [HWP, HW], [1, HW]]),
                    _ap(xsb[ch], 0, [[_pt(xsb[ch]), 128], [HW, HW], [1, HW]]),
                    mybir.ActivationFunctionType.Copy,
                )
                # reflect rows (row0 <- row2, row65 <- row63)
                nc.scalar.activation(
                    _ap(xb[ch], 1, [[ptx, 128], [1, HW]]),
                    _ap(xb[ch], 2 * HWP + 1, [[ptx, 128], [1, HW]]),
                    mybir.ActivationFunctionType.Copy,
                )
                nc.scalar.activation(
                    _ap(xb[ch], 65 * HWP + 1, [[ptx, 128], [1, HW]]),
                    _ap(xb[ch], 63 * HWP + 1, [[ptx, 128], [1, HW]]),
                    mybir.ActivationFunctionType.Copy,
                )
                # reflect cols (col0 <- col2, col65 <- col63)
                nc.scalar.activation(
                    _ap(xb[ch], 0, [[ptx, 128], [HWP, HWP]]),
                    _ap(xb[ch], 2, [[ptx, 128], [HWP, HWP]]),
                    mybir.ActivationFunctionType.Copy,
                )
                nc.scalar.activation(
                    _ap(xb[ch], 65, [[ptx, 128], [HWP, HWP]]),
                    _ap(xb[ch], 63, [[ptx, 128], [HWP, HWP]]),
                    mybir.ActivationFunctionType.Copy,
                )

            # ================ AllToAll: core c receives, from every rank r,
            # rank r's o-slice of sample c's dynamic weights.
            nc.gpsimd.collective_compute(
                "AllToAll",
                mybir.AluOpType.bypass,
                replica_groups=[list(range(N))],
                ins=[agin[:].opt()],
                outs=[agout[:].opt()],
            )



            # ================ stage B + stage C, chunk-pipelined
            # D[ch][q(part), m2*10+t(t<9) | m2*10+9=pk]
            D = [sb.tile([128, 40], BF16, tag=f"D{ch}", name=f"D{ch}") for ch in range(4)]
            PBb = [sb.tile([128, 1], BF16, tag=f"PBb{ch}", name=f"PBb{ch}") for ch in range(4)]
            for ch in range(4):
                ptD = _pt(D[ch])
                ptPB = _pt(PBb[ch])
                for half in range(2):
                    r = 2 * ch + half
                    nc.sync.dma_start(
                        out=_ap(D[ch], half * 64 * ptD, [[ptD, 64], [1, 40]]),
                        in_=_ap(agout[:], r * BLK, [[40, 64], [1, 40]]),
                    )
                    nc.sync.dma_start(
                        out=_ap(PBb[ch], half * 64 * ptPB, [[ptPB, 64], [1, 1]]),
                        in_=_ap(agout[:], r * BLK + BPB, [[1, 64], [1, 1]]),
                    )

            S = [sb.tile([128, 9 * 128], BF16, tag=f"S{ch}", name=f"S{ch}") for ch in range(4)]
            PBf = [sb.tile([128, 1], F32, tag=f"PBf{ch}", name=f"PBf{ch}") for ch in range(4)]

            def build_S(ch):
                ptD = _pt(D[ch])
                nc.vector.tensor_copy(PBf[ch][:, :], PBb[ch][:, :])
                PKf = sb.tile([128, 4], F32, tag="PKf", name="PKf")
                nc.vector.tensor_copy(PKf[:, :], _ap(D[ch], 9, [[ptD, 128], [10, 4]]))
                dp = ps.tile([128, 144], F32, tag="sA", name="sA")
                for m2 in range(4):
                    nc.tensor.matmul(
                        dp[:, m2 * 36:(m2 + 1) * 36],
                        _ap(wbC, OFF_PERM + m2 * 128, [[ptC, 128], [1, 128]]),
                        _ap(D[ch], 0, [[ptD, 128], [10, 4], [1, 9]]),
                        start=True,
                        stop=True,
                    )
                wef = sb.tile([128, 64], F32, tag="wef", name="wef")
                tmp = sb.tile([128, 36], F32, tag="weftmp", name="weftmp")
                nc.vector.tensor_scalar_mul(
                    wef[:, 0:36], dp[:, 0:36], PKf[:, 0:1]
                )
                for m2 in range(1, 4):
                    nc.vector.tensor_scalar_mul(
                        tmp[:, :], dp[:, m2 * 36:(m2 + 1) * 36], PKf[:, m2:m2 + 1]
                    )
                    nc.vector.tensor_add(wef[:, 0:36], wef[:, 0:36], tmp[:, :])
                # expand W_eff -> block-diag S via PE select-matmuls + mask
                tp = ps.tile([36, 128], F32, tag="sA", name="sA")
                nc.tensor.matmul(
                    tp[:, :], wef[:, 0:36],
                    _ap(cf, CF_IDENT, [[ptF, 128], [1, 128]]),
                    is_transpose=True, start=True, stop=True,
                )
                wefT = sb.tile([36, 128], BF16, tag="wefT", name="wefT")
                nc.vector.tensor_copy(wefT[:, :], tp[:, :])
                for t in range(9):
                    sps = ps.tile([128, 128], F32, tag="sB", name="sB")
                    nc.tensor.matmul(
                        sps[:, :],
                        selsb[:, t * 128:(t + 1) * 128],
                        wefT[:, :],
                        start=True, stop=True,
                    )
                    nc.vector.tensor_tensor(
                        S[ch][:, t * 128:(t + 1) * 128], sps[:, :],
                        _ap(cf, CF_MASK, [[ptF, 128], [1, 128]]),
                        op=mybir.AluOpType.mult,
                    )

            build_S(0)
            for ch in range(4):
                if ch + 1 < 4:
                    # build next chunk's S while this chunk's conv streams,
                    # so its vector-dependent steps never bubble the PE
                    build_S(ch + 1)
                ptx = _pt(xb[ch])
                osb = sbo.tile([128, HW * HW], BF16, tag="osb", name="osb")
                for sub in range(8):
                    cps = psc.tile([128, 512], F32, tag="cps", name="cps")
                    r0 = sub * 8
                    for tap in range(9):
                        di, dj = tap // 3, tap % 3
                        rhs = _ap(xb[ch], (r0 + di) * HWP + dj,
                                  [[ptx, 128], [HWP, 8], [1, HW]])
                        nc.tensor.matmul(
                            cps[:, :],
                            S[ch][:, tap * 128:(tap + 1) * 128],
                            rhs,
                            start=(tap == 0),
                            stop=(tap == 8),
                        )
                    # alternate PSUM drains between ACT and DVE so neither
                    # engine's throughput limits PSUM bank recycling
                    if sub % 2 == 0:
                        nc.scalar.activation(
                            osb[:, r0 * HW:(r0 + 8) * HW], cps[:, :],
                            mybir.ActivationFunctionType.Identity,
                            bias=PBf[ch][:, 0:1],
                        )
                    else:
                        nc.vector.tensor_scalar_add(
                            osb[:, r0 * HW:(r0 + 8) * HW], cps[:, :],
                            PBf[ch][:, 0:1],
                        )
                nc.scalar.dma_start(
                    out=out[ch * 128:(ch + 1) * 128, :], in_=osb[:, :]
                )

    _split_excess_waits(nc)
    return nc


_NC_CACHE = {}


def _get_nc():
    if "nc" not in _NC_CACHE:
        _NC_CACHE["nc"] = build_nc()
    return _NC_CACHE["nc"]


def make_in_maps(inputs):
    """Host-side shard/layout prep (pure layout: transpose/reshape/slice)."""
    style = np.asarray(inputs["style_encoding"], np.float32)
    pred = np.asarray(inputs["predicted"], np.float32)
    w1 = np.asarray(inputs["dw1_w"], np.float32).reshape(512, 512)
    w2 = np.asarray(inputs["dw2_w"], np.float32).reshape(2048, 512, 2, 2)
    pk1 = np.asarray(inputs["pk1_w"], np.float32).reshape(512, 512)
    pk2 = np.asarray(inputs["pk2_w"], np.float32).reshape(2048, 512)
    pb1 = np.asarray(inputs["pb1_w"], np.float32).reshape(512, 512)
    pb2 = np.asarray(inputs["pb2_w"], np.float32).reshape(512, 512)

    def blk128(mat_t):
        # [512, W] (row = input-ch) -> [128, 4*W] with block it at cols it*W
        W = mat_t.shape[1]
        return mat_t.reshape(4, 128, W).transpose(1, 0, 2).reshape(128, 4 * W)

    st_all = np.ascontiguousarray(
        style.transpose(1, 0, 2, 3).reshape(512, N * NPOS)
    )
    w1A = blk128(np.ascontiguousarray(w1.T))
    pk1A = blk128(np.ascontiguousarray(pk1.T))
    pb1A = blk128(np.ascontiguousarray(pb1.T))
    w2t_full = (
        w2.reshape(2048, 4, 128, 2, 2)
        .transpose(1, 3, 4, 2, 0)          # [it, di, dj, 128, o]
        .reshape(16, 128, 2048)
    )
    pk2t_full = np.ascontiguousarray(pk2.T).reshape(4, 128, 2048)
    pb2t_full = np.ascontiguousarray(pb2.T).reshape(4, 128, 512)

    permm = np.zeros((4, 128, 128), np.float32)
    for m2 in range(4):
        for p in range(128):
            permm[m2, 4 * (p // 4) + m2, p] = 1.0
    permA = permm.transpose(1, 0, 2).reshape(128, 512)
    identm = np.eye(128, dtype=np.float32)
    selm = np.zeros((36, 9, 128), np.float32)
    for t in range(9):
        for p in range(128):
            selm[(p % 4) * 9 + t, t, p] = 1.0
    selm = selm.reshape(36, 9 * 128).astype(BF16_NP)
    maskm = np.zeros((128, 128), np.float32)
    for p in range(128):
        for col in range(128):
            if p // 4 == col // 4:
                maskm[p, col] = 1.0

    wbA = np.hstack([
        st_all.reshape(4, 128, N * NPOS).transpose(1, 0, 2).reshape(128, 512),
        w1A,
    ]).astype(BF16_NP)
    b1c = np.asarray(inputs["dw1_b"], np.float32).reshape(4, 128).T
    bk1c = np.asarray(inputs["pk1_b"], np.float32).reshape(4, 128).T
    bb1c = np.asarray(inputs["pb1_b"], np.float32).reshape(4, 128).T

    in_maps = []
    for c in range(N):
        w2c = w2t_full[:, :, c * OSL:(c + 1) * OSL]       # [16,128,256]
        wbB = w2c.transpose(1, 0, 2).reshape(128, WBB_W).astype(BF16_NP)
        pk2c = pk2t_full[:, :, c * PKSL:(c + 1) * PKSL]   # [4,128,256]
        pb2c = pb2t_full[:, :, c * PBSL:(c + 1) * PBSL]   # [4,128,64]
        wbC = np.hstack([
            pk1A, pb1A,
            pk2c.transpose(1, 0, 2).reshape(128, 1024),
            pb2c.transpose(1, 0, 2).reshape(128, 256),
            permA,
        ]).astype(BF16_NP)
        b2c = np.asarray(inputs["dw2_b"], np.float32)[c * OSL:(c + 1) * OSL]
        bk2c = np.asarray(inputs["pk2_b"], np.float32)[c * PKSL:(c + 1) * PKSL]
        bb2c = np.asarray(inputs["pb2_b"], np.float32)[c * PBSL:(c + 1) * PBSL]
        bb2col = np.zeros((128, 1), np.float32)
        bb2col[:64, 0] = bb2c
        cf = np.hstack([
            b1c,
            b2c.reshape(2, 128).T,
            bk1c, bb1c,
            bk2c.reshape(2, 128).T,
            bb2col,
            identm, maskm,
        ]).astype(np.float32)
        assert cf.shape[1] == CF_W
        m = {
            "wbA": wbA,
            "wbB": np.ascontiguousarray(wbB),
            "wbC": np.ascontiguousarray(wbC),
            "cf": np.ascontiguousarray(cf),
            "selm": selm,
            "xin": np.ascontiguousarray(pred[c].reshape(512, HW * HW)),
        }
        in_maps.append(m)
    return in_maps


def kernel(**inputs):
    install_profile_shim()
    from concourse.bass_utils import run_bass_kernel_spmd

    nc = _get_nc()
    in_maps = make_in_maps(inputs)
    res = run_bass_kernel_spmd(nc, in_maps, core_ids=list(range(N)))
    outs = [np.asarray(res.results[c]["out"]).astype(np.float32).reshape(COUT, HW, HW)
            for c in range(N)]
    return np.stack(outs, axis=0)


# revision 22
# speedup vs baseline: 1.0243x; 1.0243x over previous
"""AdaConv Trainium2 kernel — 8-core SPMD, data-parallel over batch.

Per core c (sample c):
  Stage A: kernel-prediction net for ALL 8 samples, layer-2 weights sharded
           by output channel across cores; AllToAll redistributes so each
           core ends with the full dynamic weights for its own sample.
  Stage B: build fused per-sample conv weights W_eff = PK o D and expand
           them into block-diagonal stationary matrices S via PE select
           matmuls + mask.
  Stage C: main grouped conv (128 groups of 4->4 ch, 3x3, reflect pad) as
           9 PSUM-accumulated bf16 matmuls per 128-channel chunk.

Perf structure (v3):
  - All kernel-prediction weights host-packed into 3 bf16 tensors in final
    SBUF layout -> 3 contiguous DMAs (few hundred descriptors) instead of
    thousands of tiny strided ones. Biases/constants packed into one f32
    tensor.
  - The big `predicted` input loads contiguously on the ACT HWDGE ring;
    pad+f32->bf16 convert on-chip (scalar engine), overlapped with the
    collective.
  - AllToAll payload in bf16 with dw+pk interleaved per channel so the
    write is runs-of-10 and the read is one contiguous 40-element run per
    channel.
  - Stage B matmuls in bf16; output stored bf16 (host converts to f32).
"""
import sys
import types

sys.path.insert(0, "/opt/trn_rl_repo")

import numpy as np
import ml_dtypes

import concourse.bass as bass
import concourse.mybir as mybir

N = 8          # batch == cores
CIN = 512
COUT = 512
HW = 64        # spatial
HWP = 66       # padded
NPOS = 16      # style spatial 4x4
OSL = 2048 // N      # dw2 out-channel slice per core (256)
PKSL = 2048 // N     # pk2 slice (256)
PBSL = 512 // N      # pb2 slice (64)
# AllToAll per-rank block: [ch (256): 9 dw + 1 pk | pb 64]
BLK = OSL * 10 + PBSL      # 2624
BPB = OSL * 10             # 2560
AG_SZ = N * BLK            # 20992

# wbA columns (bf16): [st 512 | w1 2048]
OFF_ST = 0
OFF_W1 = 512
WBA_W = 2560
# wbB columns (bf16): [w2 4096]
WBB_W = 16 * OSL           # 4096
# wbC columns (bf16): [pk1 2048 | pb1 2048 | pk2 1024 | pb2 256 | perm 512]
OFF_PK1 = 0
OFF_PB1 = 2048
OFF_PK2 = 4096
OFF_PB2 = 5120
OFF_PERM = 5376
WBC_W = 5888
# cf columns (f32): [b1 4 | b2 2 | bk1 4 | bb1 4 | bk2 2 | bb2 1 | ident 128 | mask 128]
CF_B1 = 0
CF_B2 = 4
CF_BK1 = 6
CF_BB1 = 10
CF_BK2 = 14
CF_BB2 = 16
CF_IDENT = 17
CF_MASK = 145
CF_W = 273

F32 = mybir.dt.float32
BF16 = mybir.dt.bfloat16
BF16_NP = ml_dtypes.bfloat16


# ---------------------------------------------------------------- tile patch
def _install_tile_patch():
    """walrus here rejects Drain instructions with >1 sync-wait; spread the
    Tile tail-drain waits over individual SP nops."""
    import concourse.tile as tile_mod
    from concourse.vector_clock import ScopedClock

    def _patched(self, tick_clock, wait_clock):
        nc = self.nc
        drain_inst = nc.sync.drain()
        wait_clock.add_sem_waits(
            drain_inst.ins, ScopedClock({None: tick_clock.global_clock})
        )
        waits = list(drain_inst.ins.sync_info.on_wait or [])
        if len(waits) > 1:
            drain_inst.ins.sync_info.on_wait = waits[:1]
            for w in waits[1:]:
                nop = nc.sync.nop(nofuse=True, hint="tail_wait_split")
                if nop.ins.sync_info is None:
                    nop.ins.sync_info = mybir.SyncInfo(on_wait=[w], on_update=[])
                else:
                    nop.ins.sync_info.on_wait = [w]
        nc.all_engine_barrier()
        assert self.sems is not None
        popped = nc._tile_sem_poison_stack.pop()
        assert popped is self._sem_poison
        nc.clear_and_free_semaphores(list(self.sems.allocated().values()))
        nc.all_engine_barrier()

    tile_mod.TileContext._drain_and_barrier = _patched


_install_tile_patch()
from concourse.tile import TileContext  # noqa: E402


def install_profile_shim():
    """antenv.axon_hooks is missing from this image; recreate it so
    run_bass_kernel_spmd(trace=True) can capture NTFF profiles."""
    if "antenv.axon_hooks" in sys.modules:
        return
    import antenv

    mod = types.ModuleType("antenv.axon_hooks")
    mod._hook = None
    mod.set_axon_ntff_profile_hook = lambda h: setattr(mod, "_hook", h)
    mod.get_axon_ntff_profile_hook = lambda: mod._hook
    sys.modules["antenv.axon_hooks"] = mod
    antenv.axon_hooks = mod
    try:
        if "/root/.axon_site" not in sys.path:
            sys.path.insert(0, "/root/.axon_site")
        from trn_agent_boot.trn_boot import _ntff_profile_via_ctypes

        hook = _ntff_profile_via_ctypes("/opt/axon/libaxon_pjrt.so")
        mod.set_axon_ntff_profile_hook(hook)
    except Exception:
        pass


def _ap(t_ap, offset, dims):
    """Custom flat AP over a tile's underlying tensor."""
    return bass.AP(t_ap.tensor, offset, [list(d) for d in dims])


def _pt(t):
    """Physical partition pitch (elements) of a tile."""
    return t[:, :].ap[0][0]


def _split_excess_waits(nc, max_waits=1):
    """This walrus build rejects instructions carrying more than ~1 sync-wait.
    Move excess waits onto same-engine NoOps inserted just before."""
    n_split = 0
    for f in nc.m.functions:
        for bb in f.blocks:
            newlist = []
            for inst in bb.instructions:
                si = getattr(inst, "sync_info", None)
                if si is not None and si.on_wait and len(si.on_wait) > max_waits:
                    waits = list(si.on_wait)
                    for k, w in enumerate(waits[max_waits:]):
                        nop = mybir.InstNoOp(
                            name=f"{inst.name}_ws{k}",
                            engine=inst.engine,
                            bass_nofuse=True,
                            sync_info=mybir.SyncInfo(on_wait=[w], on_update=[]),
                        )
                        newlist.append(nop)
                        n_split += 1
                    si.on_wait = waits[:max_waits]
                newlist.append(inst)
            try:
                bb.instructions[:] = newlist
            except TypeError:
                bb.set_instructions(newlist)
    return n_split


def build_nc():
    nc = bass.Bass(target_bir_lowering=False)

    wbA_p = nc.declare_dram_parameter("wbA", [128, WBA_W], BF16, isOutput=False)
    wbB_p = nc.declare_dram_parameter("wbB", [128, WBB_W], BF16, isOutput=False)
    wbC_p = nc.declare_dram_parameter("wbC", [128, WBC_W], BF16, isOutput=False)
    cf_p = nc.declare_dram_parameter("cf", [128, CF_W], F32, isOutput=False)
    selm_p = nc.declare_dram_parameter("selm", [36, 9 * 128], BF16, isOutput=False)
    xin = nc.declare_dram_parameter("xin", [CIN, HW * HW], F32, isOutput=False)
    out = nc.declare_dram_parameter("out", [COUT, HW * HW], BF16, isOutput=True)

    with TileContext(nc) as tc:
        with (
            tc.tile_pool(name="sb", bufs=1) as sb,
            tc.tile_pool(name="sbx", bufs=1) as sbx,
            tc.tile_pool(name="sbo", bufs=2) as sbo,
            tc.tile_pool(name="ps", bufs=2, space="PSUM") as ps,
            tc.tile_pool(name="psc", bufs=4, space="PSUM") as psc,
            tc.tile_pool(name="dram", bufs=1, space="DRAM") as dram,
        ):
            # ================ SP-ring loads: packed weights/constants
            wbA = sb.tile([128, WBA_W], BF16, tag="wbA", name="wbA")
            nc.sync.dma_start(out=wbA[:, :], in_=wbA_p[:, :])
            wbB = sb.tile([128, WBB_W], BF16, tag="wbB", name="wbB")
            nc.sync.dma_start(out=wbB[:, :], in_=wbB_p[:, :])
            wbC = sb.tile([128, WBC_W], BF16, tag="wbC", name="wbC")
            nc.sync.dma_start(out=wbC[:, :], in_=wbC_p[:, :])
            cf = sb.tile([128, CF_W], F32, tag="cf", name="cf")
            nc.sync.dma_start(out=cf[:, :], in_=cf_p[:, :])
            selsb = sb.tile([36, 9 * 128], BF16, tag="selsb", name="selsb")
            nc.sync.dma_start(out=selsb[:, :], in_=selm_p[:, :])
            ptA, ptB, ptC, ptF = _pt(wbA), _pt(wbB), _pt(wbC), _pt(cf)

            def cfb(col, parts=128):
                return _ap(cf, col, [[ptF, parts], [1, 1]])

            # xsb input tiles are loaded on the SYNC ring AFTER the agin
            # writes (below): a dma_start occupies its issuing sequencer
            # until descriptors drain (~6us per 2MB chunk), and the 8MB of
            # Q_X packets would otherwise stall the tiny agin packets that
            # gate the collective. Issued post-agin, they fill the CC
            # firmware+skew dead time instead.
            xsb = [sbx.tile([128, HW * HW], F32, tag=f"xsb{ch}", name=f"xsb{ch}")
                   for ch in range(4)]

            # ================ stage A: layer-1 (h = lrelu(W1 s + b1))
            h = [sb.tile([128, N * NPOS], BF16, tag=f"h{ot}", name=f"h{ot}") for ot in range(4)]
            for ot in range(4):
                hp = ps.tile([128, N * NPOS], F32, tag="sA", name="sA")
                for it in range(4):
                    nc.tensor.matmul(
                        hp[:, :],
                        _ap(wbA, OFF_W1 + it * CIN + ot * 128, [[ptA, 128], [1, 128]]),
                        _ap(wbA, OFF_ST + it * 128, [[ptA, 128], [1, 128]]),
                        start=(it == 0),
                        stop=(it == 3),
                    )
                nc.scalar.activation(
                    h[ot][:, :], hp[:, :], mybir.ActivationFunctionType.Identity,
                    bias=cfb(CF_B1 + ot),
                )
                zt = sb.tile([128, N * NPOS], BF16, tag="zt", name="zt")
                nc.vector.tensor_scalar_mul(zt[:, :], h[ot][:, :], 0.01)
                nc.vector.tensor_max(h[ot][:, :], h[ot][:, :], zt[:, :])

            # ---------------- stage A: dw2 slice for all samples
            agin = dram.tile([AG_SZ], BF16)
            agout = dram.tile([AG_SZ], BF16)
            dppk = [sb.tile([128, 80], BF16, tag=f"dppk{o2}", name=f"dppk{o2}")
                    for o2 in range(2)]
            for o2 in range(2):
                dps = ps.tile([128, N * 9], F32, tag="sA", name="sA")
                for kt in range(16):
                    it, tap = kt // 4, kt % 4
                    di, dj = tap // 2, tap % 2
                    rhs = _ap(h[it], di * 4 + dj,
                              [[_pt(h[it]), 128], [NPOS, N], [4, 3], [1, 3]])
                    nc.tensor.matmul(
                        dps[:, :],
                        _ap(wbB, kt * OSL + o2 * 128, [[ptB, 128], [1, 128]]),
                        rhs,
                        start=(kt == 0),
                        stop=(kt == 15),
                    )
                # dw taps -> cols {n*10+0..8} of dppk
                nc.scalar.activation(
                    _ap(dppk[o2], 0, [[_pt(dppk[o2]), 128], [10, N], [1, 9]]),
                    _ap(dps, 0, [[_pt(dps), 128], [9, N], [1, 9]]),
                    mybir.ActivationFunctionType.Identity,
                    bias=cfb(CF_B2 + o2),
                )

            # ---------------- stage A: pooled-style path (pk / pb)
            sp = [sb.tile([128, N], BF16, tag=f"sp{i}", name=f"sp{i}") for i in range(4)]
            spf = sb.tile([128, N], F32, tag="spf", name="spf")
            for i in range(4):
                nc.vector.tensor_reduce(
                    spf[:, :],
                    _ap(wbA, OFF_ST + i * 128, [[ptA, 128], [NPOS, N], [1, NPOS]]),
                    axis=mybir.AxisListType.X,
                    op=mybir.AluOpType.add,
                )
                nc.vector.tensor_scalar_mul(sp[i][:, :], spf[:, :], 1.0 / NPOS)

            def layer1(off, bcol, tagp):
                acts = []
                for ot in range(4):
                    ap_ = ps.tile([128, N], F32, tag="sA", name="sA")
                    for it in range(4):
                        nc.tensor.matmul(
                            ap_[:, :],
                            _ap(wbC, off + it * CIN + ot * 128, [[ptC, 128], [1, 128]]),
                            sp[it][:, :],
                            start=(it == 0),
                            stop=(it == 3),
                        )
                    a = sb.tile([128, N], BF16, tag=f"{tagp}a{ot}", name=f"{tagp}a{ot}")
                    nc.scalar.activation(
                        a[:, :], ap_[:, :], mybir.ActivationFunctionType.Identity,
                        bias=cfb(bcol + ot),
                    )
                    zt2 = sb.tile([128, N], BF16, tag="zt2", name="zt2")
                    nc.vector.tensor_scalar_mul(zt2[:, :], a[:, :], 0.01)
                    nc.vector.tensor_max(a[:, :], a[:, :], zt2[:, :])
                    acts.append(a)
                return acts

            a1 = layer1(OFF_PK1, CF_BK1, "pk1")
            c1 = layer1(OFF_PB1, CF_BB1, "pb1")

            for o2 in range(2):
                pp = ps.tile([128, N], F32, tag="sA", name="sA")
                for it in range(4):
                    nc.tensor.matmul(
                        pp[:, :],
                        _ap(wbC, OFF_PK2 + it * PKSL + o2 * 128, [[ptC, 128], [1, 128]]),
                        a1[it][:, :],
                        start=(it == 0),
                        stop=(it == 3),
                    )
                # pk -> col {n*10+9} of dppk
                nc.scalar.activation(
                    _ap(dppk[o2], 9, [[_pt(dppk[o2]), 128], [10, N]]),
                    pp[:, :],
                    mybir.ActivationFunctionType.Identity,
                    bias=cfb(CF_BK2 + o2),
                )
                # agin[n*BLK + (o2*128+p)*10 + pos] = dppk[o2][p, n*10+pos]
                nc.sync.dma_start(
                    out=_ap(agin[:], o2 * 128 * 10, [[10, 128], [BLK, N], [1, 10]]),
                    in_=_ap(dppk[o2], 0, [[_pt(dppk[o2]), 128], [1, 80]]),
                )

            pbp = ps.tile([64, N], F32, tag="sA", name="sA")
            for it in range(4):
                nc.tensor.matmul(
                    pbp[:, :],
                    _ap(wbC, OFF_PB2 + it * PBSL, [[ptC, 128], [1, PBSL]]),
                    c1[it][:, :],
                    start=(it == 0),
                    stop=(it == 3),
                )
            pbc = sb.tile([64, 32], BF16, tag="pbc", name="pbc")
            nc.scalar.activation(
                pbc[:, 0:8], pbp[:, :], mybir.ActivationFunctionType.Identity,
                bias=cfb(CF_BB2, 64),
            )
            nc.sync.dma_start(
                out=_ap(agin[:], BPB, [[1, 64], [BLK, N]]),
                in_=_ap(pbc, 0, [[_pt(pbc), 64], [1, N]]),
            )

            # big input loads on the SYNC ring, after the agin path is fully
            # dispatched; they fill the collective firmware/skew dead time
            for ch in range(4):
                nc.sync.dma_start(
                    out=xsb[ch][:, :], in_=xin[ch * 128:(ch + 1) * 128, :]
                )



            # ================ pad + f32->bf16 convert (overlaps collective)
            # All pad work on the scalar (ACT) queue: a late xsb chunk never
            # blocks stage-B/C vector work or the conv PSUM drain.
            xb = [sbx.tile([128, 4384], BF16, tag=f"xb{ch}", name=f"xb{ch}")
                  for ch in range(4)]
            for ch in range(4):
                ptx = _pt(xb[ch])
                nc.scalar.activation(
                    _ap(xb[ch], HWP + 1, [[ptx, 128], [HWP, HW], [1, HW]]),
                    _ap(xsb[ch], 0, [[_pt(xsb[ch]), 128], [HW, HW], [1, HW]]),
                    mybir.ActivationFunctionType.Copy,
                )
                # reflect rows (row0 <- row2, row65 <- row63)
                nc.scalar.activation(
                    _ap(xb[ch], 1, [[ptx, 128], [1, HW]]),
                    _ap(xb[ch], 2 * HWP + 1, [[ptx, 128], [1, HW]]),
                    mybir.ActivationFunctionType.Copy,
                )
                nc.scalar.activation(
                    _ap(xb[ch], 65 * HWP + 1, [[ptx, 128], [1, HW]]),
                    _ap(xb[ch], 63 * HWP + 1, [[ptx, 128], [1, HW]]),
                    mybir.ActivationFunctionType.Copy,
                )
                # reflect cols (col0 <- col2, col65 <- col63)
                nc.scalar.activation(
                    _ap(xb[ch], 0, [[ptx, 128], [HWP, HWP]]),
                    _ap(xb[ch], 2, [[ptx, 128], [HWP, HWP]]),
                    mybir.ActivationFunctionType.Copy,
                )
                nc.scalar.activation(
                    _ap(xb[ch], 65, [[ptx, 128], [HWP, HWP]]),
                    _ap(xb[ch], 63, [[ptx, 128], [HWP, HWP]]),
                    mybir.ActivationFunctionType.Copy,
                )

            # ================ AllToAll: core c receives, from every rank r,
            # rank r's o-slice of sample c's dynamic weights.
            nc.gpsimd.collective_compute(
                "AllToAll",
                mybir.AluOpType.bypass,
                replica_groups=[list(range(N))],
                ins=[agin[:].opt()],
                outs=[agout[:].opt()],
            )



            # ================ stage B + stage C, chunk-pipelined
            # D[ch][q(part), m2*10+t(t<9) | m2*10+9=pk]
            D = [sb.tile([128, 40], BF16, tag=f"D{ch}", name=f"D{ch}") for ch in range(4)]
            PBb = [sb.tile([128, 1], BF16, tag=f"PBb{ch}", name=f"PBb{ch}") for ch in range(4)]
            for ch in range(4):
                ptD = _pt(D[ch])
                ptPB = _pt(PBb[ch])
                for half in range(2):
                    r = 2 * ch + half
                    nc.sync.dma_start(
                        out=_ap(D[ch], half * 64 * ptD, [[ptD, 64], [1, 40]]),
                        in_=_ap(agout[:], r * BLK, [[40, 64], [1, 40]]),
                    )
                    nc.sync.dma_start(
                        out=_ap(PBb[ch], half * 64 * ptPB, [[ptPB, 64], [1, 1]]),
                        in_=_ap(agout[:], r * BLK + BPB, [[1, 64], [1, 1]]),
                    )

            S = [sb.tile([128, 9 * 128], BF16, tag=f"S{ch}", name=f"S{ch}") for ch in range(4)]
            PBf = [sb.tile([128, 1], F32, tag=f"PBf{ch}", name=f"PBf{ch}") for ch in range(4)]

            def build_S(ch):
                ptD = _pt(D[ch])
                nc.vector.tensor_copy(PBf[ch][:, :], PBb[ch][:, :])
                PKf = sb.tile([128, 4], F32, tag="PKf", name="PKf")
                nc.vector.tensor_copy(PKf[:, :], _ap(D[ch], 9, [[ptD, 128], [10, 4]]))
                dp = ps.tile([128, 144], F32, tag="sA", name="sA")
                for m2 in range(4):
                    nc.tensor.matmul(
                        dp[:, m2 * 36:(m2 + 1) * 36],
                        _ap(wbC, OFF_PERM + m2 * 128, [[ptC, 128], [1, 128]]),
                        _ap(D[ch], 0, [[ptD, 128], [10, 4], [1, 9]]),
                        start=True,
                        stop=True,
                    )
                wef = sb.tile([128, 64], F32, tag="wef", name="wef")
                tmp = sb.tile([128, 36], F32, tag="weftmp", name="weftmp")
                nc.vector.tensor_scalar_mul(
                    wef[:, 0:36], dp[:, 0:36], PKf[:, 0:1]
                )
                for m2 in range(1, 4):
                    nc.vector.tensor_scalar_mul(
                        tmp[:, :], dp[:, m2 * 36:(m2 + 1) * 36], PKf[:, m2:m2 + 1]
                    )
                    nc.vector.tensor_add(wef[:, 0:36], wef[:, 0:36], tmp[:, :])
                # expand W_eff -> block-diag S via PE select-matmuls + mask
                tp = ps.tile([36, 128], F32, tag="sA", name="sA")
                nc.tensor.matmul(
                    tp[:, :], wef[:, 0:36],
                    _ap(cf, CF_IDENT, [[ptF, 128], [1, 128]]),
                    is_transpose=True, start=True, stop=True,
                )
                wefT = sb.tile([36, 128], BF16, tag="wefT", name="wefT")
                nc.vector.tensor_copy(wefT[:, :], tp[:, :])
                for t in range(9):
                    sps = ps.tile([128, 128], F32, tag="sB", name="sB")
                    nc.tensor.matmul(
                        sps[:, :],
                        selsb[:, t * 128:(t + 1) * 128],
                        wefT[:, :],
                        start=True, stop=True,
                    )
                    nc.vector.tensor_tensor(
                        S[ch][:, t * 128:(t + 1) * 128], sps[:, :],
                        _ap(cf, CF_MASK, [[ptF, 128], [1, 128]]),
                        op=mybir.AluOpType.mult,
                    )

            build_S(0)
            for ch in range(4):
                if ch + 1 < 4:
                    # build next chunk's S while this chunk's conv streams,
                    # so its vector-dependent steps never bubble the PE
                    build_S(ch + 1)
                ptx = _pt(xb[ch])
                osb = sbo.tile([128, HW * HW], BF16, tag="osb", name="osb")
                for sub in range(8):
                    cps = psc.tile([128, 512], F32, tag="cps", name="cps")
                    r0 = sub * 8
                    for tap in range(9):
                        di, dj = tap // 3, tap % 3
                        rhs = _ap(xb[ch], (r0 + di) * HWP + dj,
                                  [[ptx, 128], [HWP, 8], [1, HW]])
                        nc.tensor.matmul(
                            cps[:, :],
                            S[ch][:, tap * 128:(tap + 1) * 128],
                            rhs,
                            start=(tap == 0),
                            stop=(tap == 8),
                        )
                    # alternate PSUM drains between ACT and DVE so neither
                    # engine's throughput limits PSUM bank recycling
                    if sub % 2 == 0:
                        nc.scalar.activation(
                            osb[:, r0 * HW:(r0 + 8) * HW], cps[:, :],
                            mybir.ActivationFunctionType.Identity,
                            bias=PBf[ch][:, 0:1],
                        )
                    else:
                        nc.vector.tensor_scalar_add(
                            osb[:, r0 * HW:(r0 + 8) * HW], cps[:, :],
                            PBf[ch][:, 0:1],
                        )
                nc.scalar.dma_start(
                    out=out[ch * 128:(ch + 1) * 128, :], in_=osb[:, :]
                )

    _split_excess_waits(nc)
    return nc


_NC_CACHE = {}


def _get_nc():
    if "nc" not in _NC_CACHE:
        _NC_CACHE["nc"] = build_nc()
    return _NC_CACHE["nc"]


def make_in_maps(inputs):
    """Host-side shard/layout prep (pure layout: transpose/reshape/slice)."""
    style = np.asarray(inputs["style_encoding"], np.float32)
    pred = np.asarray(inputs["predicted"], np.float32)
    w1 = np.asarray(inputs["dw1_w"], np.float32).reshape(512, 512)
    w2 = np.asarray(inputs["dw2_w"], np.float32).reshape(2048, 512, 2, 2)
    pk1 = np.asarray(inputs["pk1_w"], np.float32).reshape(512, 512)
    pk2 = np.asarray(inputs["pk2_w"], np.float32).reshape(2048, 512)
    pb1 = np.asarray(inputs["pb1_w"], np.float32).reshape(512, 512)
    pb2 = np.asarray(inputs["pb2_w"], np.float32).reshape(512, 512)

    def blk128(mat_t):
        # [512, W] (row = input-ch) -> [128, 4*W] with block it at cols it*W
        W = mat_t.shape[1]
        return mat_t.reshape(4, 128, W).transpose(1, 0, 2).reshape(128, 4 * W)

    st_all = np.ascontiguousarray(
        style.transpose(1, 0, 2, 3).reshape(512, N * NPOS)
    )
    w1A = blk128(np.ascontiguousarray(w1.T))
    pk1A = blk128(np.ascontiguousarray(pk1.T))
    pb1A = blk128(np.ascontiguousarray(pb1.T))
    w2t_full = (
        w2.reshape(2048, 4, 128, 2, 2)
        .transpose(1, 3, 4, 2, 0)          # [it, di, dj, 128, o]
        .reshape(16, 128, 2048)
    )
    pk2t_full = np.ascontiguousarray(pk2.T).reshape(4, 128, 2048)
    pb2t_full = np.ascontiguousarray(pb2.T).reshape(4, 128, 512)

    permm = np.zeros((4, 128, 128), np.float32)
    for m2 in range(4):
        for p in range(128):
            permm[m2, 4 * (p // 4) + m2, p] = 1.0
    permA = permm.transpose(1, 0, 2).reshape(128, 512)
    identm = np.eye(128, dtype=np.float32)
    selm = np.zeros((36, 9, 128), np.float32)
    for t in range(9):
        for p in range(128):
            selm[(p % 4) * 9 + t, t, p] = 1.0
    selm = selm.reshape(36, 9 * 128).astype(BF16_NP)
    maskm = np.zeros((128, 128), np.float32)
    for p in range(128):
        for col in range(128):
            if p // 4 == col // 4:
                maskm[p, col] = 1.0

    wbA = np.hstack([
        st_all.reshape(4, 128, N * NPOS).transpose(1, 0, 2).reshape(128, 512),
        w1A,
    ]).astype(BF16_NP)
    b1c = np.asarray(inputs["dw1_b"], np.float32).reshape(4, 128).T
    bk1c = np.asarray(inputs["pk1_b"], np.float32).reshape(4, 128).T
    bb1c = np.asarray(inputs["pb1_b"], np.float32).reshape(4, 128).T

    in_maps = []
    for c in range(N):
        w2c = w2t_full[:, :, c * OSL:(c + 1) * OSL]       # [16,128,256]
        wbB = w2c.transpose(1, 0, 2).reshape(128, WBB_W).astype(BF16_NP)
        pk2c = pk2t_full[:, :, c * PKSL:(c + 1) * PKSL]   # [4,128,256]
        pb2c = pb2t_full[:, :, c * PBSL:(c + 1) * PBSL]   # [4,128,64]
        wbC = np.hstack([
            pk1A, pb1A,
            pk2c.transpose(1, 0, 2).reshape(128, 1024),
            pb2c.transpose(1, 0, 2).reshape(128, 256),
            permA,
        ]).astype(BF16_NP)
        b2c = np.asarray(inputs["dw2_b"], np.float32)[c * OSL:(c + 1) * OSL]
        bk2c = np.asarray(inputs["pk2_b"], np.float32)[c * PKSL:(c + 1) * PKSL]
        bb2c = np.asarray(inputs["pb2_b"], np.float32)[c * PBSL:(c + 1) * PBSL]
        bb2col = np.zeros((128, 1), np.float32)
        bb2col[:64, 0] = bb2c
        cf = np.hstack([
            b1c,
            b2c.reshape(2, 128).T,
            bk1c, bb1c,
            bk2c.reshape(2, 128).T,
            bb2col,
            identm, maskm,
        ]).astype(np.float32)
        assert cf.shape[1] == CF_W
        m = {
            "wbA": wbA,
            "wbB": np.ascontiguousarray(wbB),
            "wbC": np.ascontiguousarray(wbC),
            "cf": np.ascontiguousarray(cf),
            "selm": selm,
            "xin": np.ascontiguousarray(pred[c].reshape(512, HW * HW)),
        }
        in_maps.append(m)
    return in_maps


def kernel(**inputs):
    install_profile_shim()
    from concourse.bass_utils import run_bass_kernel_spmd

    nc = _get_nc()
    in_maps = make_in_maps(inputs)
    res = run_bass_kernel_spmd(nc, in_maps, core_ids=list(range(N)))
    outs = [np.asarray(res.results[c]["out"]).astype(np.float32).reshape(COUT, HW, HW)
            for c in range(N)]
    return np.stack(outs, axis=0)
